# revision 13
# baseline (speedup 1.0000x reference)
"""Trainium2 Bass kernel for the Bayesian SNN problem.

Model (per reference):
  w1 = w1_mu + eps1 * exp(0.5*w1_logvar)          [2048, 4096]
  w2 = w2_mu + eps2 * exp(0.5*w2_logvar)          [4096, 1024]
  5-step LIF over batch 2048:
    mem = 0.95*mem + cur - (prev_mem > 1)
    spk = (mem > 1)
  out = sum_t spk2   -> [2048, 1024]

Strategy: pure data-parallel over batch (256 rows/core, 8 cores, no
collectives). The matmuls have no cross-timestep dependency, so each core
runs one batched GEMM over all (t, b) rows per layer; only the cheap
elementwise membrane scan is sequential in t.

Per-core layout: "hidden on partitions". GEMM1 computes
cur1[h, t*256+b] = sum_k w1[k, h] * xT[k, t*256+b] with xT resident in SBUF
(10.5 MB) and w1 generated on the fly from its mu/logvar/eps column-blocks.
The LIF scan runs on each [128, 1280] psum tile; spikes (exactly 0/1) are
stored bf16 and round-trip through DRAM. GEMM2 contracts hidden with w2
split into bf16 hi+lo parts (spk in {0,1} makes products exact, psum
accumulates fp32, and hi+lo carries ~17 mantissa bits of w2, comparable to
the fp32 accumulation-order noise). Output lands as [256, 1024] rows
directly.
"""

import numpy as np

import concourse.bass as bass
import concourse.tile as tile
from concourse import bacc, mybir
from concourse.bass_utils import run_bass_kernel_spmd

F32 = mybir.dt.float32
BF16 = mybir.dt.bfloat16
F16 = mybir.dt.float16
ALU = mybir.AluOpType
ACTF = mybir.ActivationFunctionType

P = 128
B, T, DIN, DH, DOUT = 2048, 5, 2048, 4096, 1024
NCORES = 8
BC = B // NCORES            # 256 batch rows per core
TB = T * BC                 # 1280 batched-time rows per core
KO1 = DIN // P              # 16 contraction tiles, layer 1
NT = DH // P                # 32 hidden tiles
KO2 = DH // P               # 32 contraction tiles, layer 2
BETA = 0.95
THRESH = 1.0

# fp32 moving-operand chunks covering the TB free dim
MM_CHUNKS = ((0, 512), (512, 512), (1024, 256))


def _build_nc():
    nc = bacc.Bacc(
        "TRN2",
        target_bir_lowering=False,
        debug=False,
        num_devices=NCORES,
    )

    xt = nc.dram_tensor("xt", [DIN, TB], F32, kind="ExternalInput").ap()
    w1_mu = nc.dram_tensor("w1_mu", [DIN, DH], F32, kind="ExternalInput").ap()
    w1_lv = nc.dram_tensor("w1_logvar", [DIN, DH], F32, kind="ExternalInput").ap()
    eps1 = nc.dram_tensor("eps1", [DIN, DH], F32, kind="ExternalInput").ap()
    w2_mu = nc.dram_tensor("w2_mu", [DH, DOUT], F32, kind="ExternalInput").ap()
    w2_lv = nc.dram_tensor("w2_logvar", [DH, DOUT], F32, kind="ExternalInput").ap()
    eps2 = nc.dram_tensor("eps2", [DH, DOUT], F32, kind="ExternalInput").ap()
    out = nc.dram_tensor("out", [BC, DOUT], F32, kind="ExternalOutput").ap()

    with tile.TileContext(nc) as tc:
        with tc.tile_pool(name="dram", bufs=1, space="DRAM") as dramp:
            spk1d = dramp.tile([NT, P, TB], F16)
            w2f16d = dramp.tile([KO2, P, DOUT], F16)

            # ---------------- Phase A: layer 1 ----------------
            with (
                tc.tile_pool(name="xres", bufs=1) as xp,
                tc.tile_pool(name="aw1t", bufs=3) as wp1,
                tc.tile_pool(name="aw1hl", bufs=3) as whl,
                tc.tile_pool(name="aq", bufs=8) as qp,
                tc.tile_pool(name="axs", bufs=2) as xsp,
                tc.tile_pool(name="aspk", bufs=3) as sp,
                tc.tile_pool(name="amem", bufs=2) as mp,
                tc.tile_pool(name="w2gen", bufs=2) as w2p,
                tc.tile_pool(name="apsum", bufs=2, space="PSUM") as aps,
            ):
                pend_data = {}

                def emit_w1gen_loads(n):
                    """DMA mu/logvar/eps column-block for hidden tile n."""
                    ncols = slice(n * P, (n + 1) * P)
                    w1t = wp1.tile([P, KO1, P], F32, tag="w1t")
                    nc.sync.dma_start(
                        w1t, w1_mu[:, ncols].rearrange("(o p) n -> p o n", p=P)
                    )
                    qs = []
                    for q in range(4):
                        rows = slice(q * 4 * P, (q + 1) * 4 * P)
                        lvq = qp.tile([P, 4, P], F32, tag="lv1")
                        epq = qp.tile([P, 4, P], F32, tag="ep1")
                        nc.sync.dma_start(
                            lvq,
                            w1_lv[rows, ncols].rearrange("(o p) n -> p o n", p=P),
                        )
                        nc.sync.dma_start(
                            epq,
                            eps1[rows, ncols].rearrange("(o p) n -> p o n", p=P),
                        )
                        qs.append((lvq, epq))
                    pend_data[n] = (w1t, qs)

                def emit_w1gen_compute(n):
                    """Build w1 = mu + eps*exp(0.5*lv), split bf16 hi/lo.
                    Emitted a full tile after its loads so the DVE/ACT queue
                    heads never sit blocked on the gen DMAs."""
                    w1t, qs = pend_data.pop(n)
                    for q, (lvq, epq) in enumerate(qs):
                        # flatten contiguous [P, 4, P] -> [P, 512]: 3D APs
                        # overflow the TT instruction's sync-wait encoding
                        lvf = lvq.rearrange("p a b -> p (a b)")
                        epf = epq.rearrange("p a b -> p (a b)")
                        w1f = w1t[:, q * 4 : (q + 1) * 4, :].rearrange(
                            "p a b -> p (a b)"
                        )
                        nc.scalar.activation(lvf, lvf, ACTF.Exp, scale=0.5)
                        nc.vector.scalar_tensor_tensor(
                            epf, epf, 1.0, lvf, op0=ALU.bypass, op1=ALU.mult
                        )
                        nc.gpsimd.tensor_tensor(w1f, w1f, epf, op=ALU.add)
                    w1h = whl.tile([P, KO1, P], BF16, tag="w1h")
                    w1l = whl.tile([P, KO1, P], BF16, tag="w1l")
                    w1tf = w1t.rearrange("p a b -> p (a b)")
                    w1hf = w1h.rearrange("p a b -> p (a b)")
                    w1lf = w1l.rearrange("p a b -> p (a b)")
                    nc.scalar.activation(w1hf, w1tf, ACTF.Copy)
                    nc.gpsimd.tensor_tensor(w1lf, w1tf, w1hf, op=ALU.subtract)
                    return w1h, w1l

                def emit_w2gen(o2):
                    """One k-row block of w2 -> fp16 in DRAM scratch."""
                    orows = slice(o2 * P, (o2 + 1) * P)
                    mu2t = w2p.tile([P, DOUT], F32, tag="mu2")
                    lv2t = w2p.tile([P, DOUT], F32, tag="lv2")
                    ep2t = w2p.tile([P, DOUT], F32, tag="ep2")
                    nc.sync.dma_start(mu2t, w2_mu[orows, :])
                    nc.sync.dma_start(lv2t, w2_lv[orows, :])
                    nc.sync.dma_start(ep2t, eps2[orows, :])
                    nc.scalar.activation(lv2t, lv2t, ACTF.Exp, scale=0.5)
                    nc.vector.scalar_tensor_tensor(
                        ep2t, ep2t, 1.0, lv2t, op0=ALU.bypass, op1=ALU.mult
                    )
                    nc.gpsimd.tensor_tensor(mu2t, mu2t, ep2t, op=ALU.add)
                    hi2 = w2p.tile([P, DOUT], F16, tag="hi2")
                    nc.vector.tensor_copy(hi2, mu2t)
                    nc.scalar.dma_start(w2f16d[o2], hi2)

                # x staging rides the scalar DMA queue so it never queues
                # behind the w1-gen loads on the sync queue
                XH = xp.tile([P, KO1, TB], BF16)
                XL = xp.tile([P, KO1, TB], BF16)
                for o in range(KO1):
                    xs = xsp.tile([P, TB], F32, tag="xstage")
                    nc.scalar.dma_start(xs, xt[o * P : (o + 1) * P, :])
                    nc.scalar.activation(XH[:, o, :], xs, ACTF.Copy)
                    nc.vector.scalar_tensor_tensor(
                        XL[:, o, :], xs, 1.0, XH[:, o, :],
                        op0=ALU.bypass, op1=ALU.subtract,
                    )

                # w-gen pipeline: loads run 2 tiles ahead of the matmuls,
                # compute emitted at the end of the previous-previous tile
                emit_w1gen_loads(0)
                emit_w1gen_loads(1)
                pend = {0: emit_w1gen_compute(0), 1: emit_w1gen_compute(1)}

                for n in range(NT):
                    w1h, w1l = pend.pop(n)
                    if n + 2 < NT:
                        emit_w1gen_loads(n + 2)

                    ps = aps.tile([P, TB], F32, tag="ps1")
                    for k in range(KO1):
                        for pi, (lt, rt) in enumerate(
                            ((w1h, XH), (w1h, XL), (w1l, XH))
                        ):
                            for c0, cw in MM_CHUNKS:
                                nc.tensor.matmul(
                                    ps[:, c0 : c0 + cw],
                                    lt[:, k, :],
                                    rt[:, k, c0 : c0 + cw],
                                    start=(k == 0 and pi == 0),
                                    stop=(k == KO1 - 1 and pi == 2),
                                )

                    # LIF scan over t on this hidden tile
                    spkt = sp.tile([P, TB], F16, tag="spk")
                    mem = mp.tile([P, BC], F32, tag="mem1")
                    nc.scalar.activation(mem, ps[:, 0:BC], ACTF.Copy)
                    nc.vector.tensor_scalar(
                        spkt[:, 0:BC], mem, THRESH, None, op0=ALU.is_gt
                    )
                    for t in range(1, T):
                        tsl = slice(t * BC, (t + 1) * BC)
                        psl = slice((t - 1) * BC, t * BC)
                        nc.vector.scalar_tensor_tensor(
                            mem, mem, BETA, spkt[:, psl],
                            op0=ALU.mult, op1=ALU.subtract,
                        )
                        nc.vector.scalar_tensor_tensor(
                            mem, mem, 1.0, ps[:, tsl], op0=ALU.bypass, op1=ALU.add
                        )
                        nc.vector.tensor_scalar(
                            spkt[:, tsl], mem, THRESH, None, op0=ALU.is_gt
                        )
                    # issue from DVE so the write needs no semaphore wait
                    nc.scalar.dma_start(spk1d[n], spkt)

                    # pace one w2 block per hidden tile: its DMAs never
                    # sit blocked at the head of the sync queue
                    emit_w2gen(n)

                    # gen compute for tile n+2 lands after this tile's LIF
                    # ops, so LIF never waits behind DMA-blocked gen ops
                    if n + 2 < NT:
                        pend[n + 2] = emit_w1gen_compute(n + 2)

            # ---------------- Phase B: layer 2 ----------------
            with (
                tc.tile_pool(name="w2res", bufs=1) as w2r,
                tc.tile_pool(name="bspk", bufs=3) as bp,
                tc.tile_pool(name="bstate", bufs=1) as bs,
                tc.tile_pool(name="bpsum", bufs=2, space="PSUM") as bps,
            ):
                W2F = w2r.tile([P, KO2, DOUT], F16)

                acc = bs.tile([P, 2, DOUT], F32)
                mem2 = bs.tile([P, 2, DOUT], F32)
                spk2 = bs.tile([P, 2, DOUT], F32)

                first = True
                for t in range(T):
                    for h in range(2):
                        spks = bp.tile([P, KO2, P], F16, tag="spkB")
                        coff = t * BC + h * P
                        for k2 in range(KO2):
                            nc.sync.dma_start(
                                spks[:, k2, :],
                                spk1d[k2, :, coff : coff + P],
                            )
                        if first:
                            # W2 residency loads come after the first spk
                            # block; split across two DMA queues so the
                            # first blocks' matmuls aren't stream-gated.
                            for o2 in range(KO2):
                                q = nc.scalar if o2 % 2 else nc.sync
                                q.dma_start(W2F[:, o2, :], w2f16d[o2])
                            first = False
                        ps2 = bps.tile([P, DOUT], F32, tag="ps2")
                        for k2 in range(KO2):
                            for c0 in (0, 512):
                                csl = slice(c0, c0 + 512)
                                nc.tensor.matmul(
                                    ps2[:, csl], spks[:, k2, :], W2F[:, k2, csl],
                                    start=(k2 == 0), stop=(k2 == KO2 - 1),
                                )
                        m2 = mem2[:, h, :]
                        if t == 0:
                            nc.scalar.activation(m2, ps2, ACTF.Copy)
                            nc.vector.tensor_scalar(
                                acc[:, h, :], m2, THRESH, None, op0=ALU.is_gt
                            )
                            nc.scalar.activation(spk2[:, h, :], acc[:, h, :], ACTF.Copy)
                        else:
                            nc.vector.scalar_tensor_tensor(
                                m2, m2, BETA, spk2[:, h, :],
                                op0=ALU.mult, op1=ALU.subtract,
                            )
                            nc.vector.scalar_tensor_tensor(
                                m2, m2, 1.0, ps2, op0=ALU.bypass, op1=ALU.add
                            )
                            if t < T - 1:
                                nc.vector.tensor_scalar(
                                    spk2[:, h, :], m2, THRESH, None, op0=ALU.is_gt
                                )
                            nc.vector.scalar_tensor_tensor(
                                acc[:, h, :], m2, THRESH, acc[:, h, :],
                                op0=ALU.is_gt, op1=ALU.add,
                            )
                for h in range(2):
                    nc.sync.dma_start(out[h * P : (h + 1) * P, :], acc[:, h, :])

    nc.compile()
    return nc


_NC_CACHE = None


def _get_nc():
    global _NC_CACHE
    if _NC_CACHE is None:
        _NC_CACHE = _build_nc()
    return _NC_CACHE


def _make_in_maps(inputs):
    x = np.ascontiguousarray(inputs["x"], dtype=np.float32)
    shared = {
        name: np.ascontiguousarray(inputs[name], dtype=np.float32)
        for name in ("w1_mu", "w1_logvar", "eps1", "w2_mu", "w2_logvar", "eps2")
    }
    in_maps = []
    for c in range(NCORES):
        xc = x[c * BC : (c + 1) * BC]          # [BC, T, DIN]
        xtc = np.ascontiguousarray(xc.transpose(2, 1, 0)).reshape(DIN, TB)
        in_maps.append({"xt": xtc, **shared})
    return in_maps


def _run(inputs, trace=False, **kwargs):
    nc = _get_nc()
    in_maps = _make_in_maps(inputs)
    res = run_bass_kernel_spmd(
        nc, in_maps, core_ids=list(range(NCORES)), trace=trace, **kwargs
    )
    outs = [np.asarray(res.results[c]["out"]) for c in range(NCORES)]
    full = np.concatenate(outs, axis=0).astype(np.float32)
    return full, res


def kernel(**inputs):
    full, _ = _run(inputs, trace=False)
    return full



# revision 19
# speedup vs baseline: 1.0356x; 1.0356x over previous
"""Trainium2 Bass kernel for the Bayesian SNN problem.

Model (per reference):
  w1 = w1_mu + eps1 * exp(0.5*w1_logvar)          [2048, 4096]
  w2 = w2_mu + eps2 * exp(0.5*w2_logvar)          [4096, 1024]
  5-step LIF over batch 2048:
    mem = 0.95*mem + cur - (prev_mem > 1)
    spk = (mem > 1)
  out = sum_t spk2   -> [2048, 1024]

Strategy: pure data-parallel over batch (256 rows/core, 8 cores, no
collectives). The matmuls have no cross-timestep dependency, so each core
runs one batched GEMM over all (t, b) rows per layer; only the cheap
elementwise membrane scan is sequential in t.

Per-core layout: "hidden on partitions". GEMM1 computes
cur1[h, t*256+b] = sum_k w1[k, h] * xT[k, t*256+b] with xT resident in SBUF
(10.5 MB) and w1 generated on the fly from its mu/logvar/eps column-blocks.
The LIF scan runs on each [128, 1280] psum tile; spikes (exactly 0/1) are
stored bf16 and round-trip through DRAM. GEMM2 contracts hidden with w2
split into bf16 hi+lo parts (spk in {0,1} makes products exact, psum
accumulates fp32, and hi+lo carries ~17 mantissa bits of w2, comparable to
the fp32 accumulation-order noise). Output lands as [256, 1024] rows
directly.
"""

import numpy as np

import concourse.bass as bass
import concourse.tile as tile
from concourse import bacc, mybir
from concourse.bass_utils import run_bass_kernel_spmd

F32 = mybir.dt.float32
BF16 = mybir.dt.bfloat16
F16 = mybir.dt.float16
ALU = mybir.AluOpType
ACTF = mybir.ActivationFunctionType

P = 128
B, T, DIN, DH, DOUT = 2048, 5, 2048, 4096, 1024
NCORES = 8
BC = B // NCORES            # 256 batch rows per core
TB = T * BC                 # 1280 batched-time rows per core
KO1 = DIN // P              # 16 contraction tiles, layer 1
NT = DH // P                # 32 hidden tiles
KO2 = DH // P               # 32 contraction tiles, layer 2
BETA = 0.95
THRESH = 1.0

# fp32 moving-operand chunks covering the TB free dim
MM_CHUNKS = ((0, 512), (512, 512), (1024, 256))


def _build_nc():
    nc = bacc.Bacc(
        "TRN2",
        target_bir_lowering=False,
        debug=False,
        num_devices=NCORES,
    )

    xt = nc.dram_tensor("xt", [DIN, TB], F32, kind="ExternalInput").ap()
    w1_mu = nc.dram_tensor("w1_mu", [DIN, DH], F32, kind="ExternalInput").ap()
    w1_lv = nc.dram_tensor("w1_logvar", [DIN, DH], F32, kind="ExternalInput").ap()
    eps1 = nc.dram_tensor("eps1", [DIN, DH], F32, kind="ExternalInput").ap()
    w2_mu = nc.dram_tensor("w2_mu", [DH, DOUT], F32, kind="ExternalInput").ap()
    w2_lv = nc.dram_tensor("w2_logvar", [DH, DOUT], F32, kind="ExternalInput").ap()
    eps2 = nc.dram_tensor("eps2", [DH, DOUT], F32, kind="ExternalInput").ap()
    out = nc.dram_tensor("out", [BC, DOUT], F32, kind="ExternalOutput").ap()

    with tile.TileContext(nc) as tc:
        with tc.tile_pool(name="dram", bufs=1, space="DRAM") as dramp:
            spk1d = dramp.tile([NT, P, TB], F16)
            w2f16d = dramp.tile([KO2, P, DOUT], F16)

            # ---------------- Phase A: layer 1 ----------------
            with (
                tc.tile_pool(name="xres", bufs=1) as xp,
                tc.tile_pool(name="aw1t", bufs=3) as wp1,
                tc.tile_pool(name="aw1hl", bufs=3) as whl,
                tc.tile_pool(name="aq", bufs=5) as qp,
                tc.tile_pool(name="acur", bufs=2) as curp,
                tc.tile_pool(name="axs", bufs=2) as xsp,
                tc.tile_pool(name="aspk", bufs=3) as sp,
                tc.tile_pool(name="amem", bufs=2) as mp,
                tc.tile_pool(name="w2gen", bufs=2) as w2p,
                tc.tile_pool(name="apsum", bufs=2, space="PSUM") as aps,
            ):
                pend_data = {}

                def emit_w1gen_loads(n):
                    """DMA mu/logvar/eps column-block for hidden tile n."""
                    ncols = slice(n * P, (n + 1) * P)
                    w1t = wp1.tile([P, KO1, P], F32, tag="w1t")
                    nc.sync.dma_start(
                        w1t, w1_mu[:, ncols].rearrange("(o p) n -> p o n", p=P)
                    )
                    qs = []
                    for q in range(4):
                        rows = slice(q * 4 * P, (q + 1) * 4 * P)
                        lvq = qp.tile([P, 4, P], F32, tag="lv1")
                        epq = qp.tile([P, 4, P], F32, tag="ep1")
                        nc.sync.dma_start(
                            lvq,
                            w1_lv[rows, ncols].rearrange("(o p) n -> p o n", p=P),
                        )
                        nc.sync.dma_start(
                            epq,
                            eps1[rows, ncols].rearrange("(o p) n -> p o n", p=P),
                        )
                        qs.append((lvq, epq))
                    pend_data[n] = (w1t, qs)

                def emit_w1gen_compute(n):
                    """Build w1 = mu + eps*exp(0.5*lv), split bf16 hi/lo.
                    Emitted a full tile after its loads so the DVE/ACT queue
                    heads never sit blocked on the gen DMAs."""
                    w1t, qs = pend_data.pop(n)
                    for q, (lvq, epq) in enumerate(qs):
                        # flatten contiguous [P, 4, P] -> [P, 512]: 3D APs
                        # overflow the TT instruction's sync-wait encoding
                        lvf = lvq.rearrange("p a b -> p (a b)")
                        epf = epq.rearrange("p a b -> p (a b)")
                        w1f = w1t[:, q * 4 : (q + 1) * 4, :].rearrange(
                            "p a b -> p (a b)"
                        )
                        nc.scalar.activation(lvf, lvf, ACTF.Exp, scale=0.5)
                        nc.vector.scalar_tensor_tensor(
                            epf, epf, 1.0, lvf, op0=ALU.bypass, op1=ALU.mult
                        )
                        nc.gpsimd.tensor_tensor(w1f, w1f, epf, op=ALU.add)
                    w1h = whl.tile([P, KO1, P], BF16, tag="w1h")
                    w1l = whl.tile([P, KO1, P], BF16, tag="w1l")
                    w1tf = w1t.rearrange("p a b -> p (a b)")
                    w1hf = w1h.rearrange("p a b -> p (a b)")
                    w1lf = w1l.rearrange("p a b -> p (a b)")
                    nc.scalar.activation(w1hf, w1tf, ACTF.Copy)
                    nc.gpsimd.tensor_tensor(w1lf, w1tf, w1hf, op=ALU.subtract)
                    return w1h, w1l

                def emit_w2gen(o2):
                    """One k-row block of w2 -> fp16 in DRAM scratch."""
                    orows = slice(o2 * P, (o2 + 1) * P)
                    mu2t = w2p.tile([P, DOUT], F32, tag="mu2")
                    lv2t = w2p.tile([P, DOUT], F32, tag="lv2")
                    ep2t = w2p.tile([P, DOUT], F32, tag="ep2")
                    nc.sync.dma_start(mu2t, w2_mu[orows, :])
                    nc.sync.dma_start(lv2t, w2_lv[orows, :])
                    nc.sync.dma_start(ep2t, eps2[orows, :])
                    nc.scalar.activation(lv2t, lv2t, ACTF.Exp, scale=0.5)
                    nc.vector.scalar_tensor_tensor(
                        ep2t, ep2t, 1.0, lv2t, op0=ALU.bypass, op1=ALU.mult
                    )
                    nc.gpsimd.tensor_tensor(mu2t, mu2t, ep2t, op=ALU.add)
                    hi2 = w2p.tile([P, DOUT], F16, tag="hi2")
                    nc.vector.tensor_copy(hi2, mu2t)
                    nc.scalar.dma_start(w2f16d[o2], hi2)

                # w-gen for the first two tiles comes first so the ACT/DVE
                # queues aren't head-of-line blocked behind the x staging
                emit_w1gen_loads(0)
                emit_w1gen_loads(1)
                pend = {0: emit_w1gen_compute(0), 1: emit_w1gen_compute(1)}

                # x staging rides the scalar DMA queue so it never queues
                # behind the w1-gen loads on the sync queue
                XH = xp.tile([P, KO1, TB], BF16)
                XL = xp.tile([P, KO1, TB], BF16)
                for o in range(KO1):
                    xs = xsp.tile([P, TB], F32, tag="xstage")
                    nc.scalar.dma_start(xs, xt[o * P : (o + 1) * P, :])
                    nc.scalar.activation(XH[:, o, :], xs, ACTF.Copy)
                    nc.vector.scalar_tensor_tensor(
                        XL[:, o, :], xs, 1.0, XH[:, o, :],
                        op0=ALU.bypass, op1=ALU.subtract,
                    )

                for n in range(NT):
                    w1h, w1l = pend.pop(n)
                    if n + 2 < NT:
                        emit_w1gen_loads(n + 2)

                    ps = aps.tile([P, TB], F32, tag="ps1")
                    for k in range(KO1):
                        for pi, (lt, rt) in enumerate(
                            ((w1h, XH), (w1h, XL), (w1l, XH))
                        ):
                            for c0, cw in MM_CHUNKS:
                                nc.tensor.matmul(
                                    ps[:, c0 : c0 + cw],
                                    lt[:, k, :],
                                    rt[:, k, c0 : c0 + cw],
                                    start=(k == 0 and pi == 0),
                                    stop=(k == KO1 - 1 and pi == 2),
                                )

                    # One ACT copy drains the psum tile to SBUF so it
                    # recycles in ~1us; the serial LIF chain then runs off
                    # the copy without gating the tensor engine.
                    cur = curp.tile([P, TB], F32, tag="cur1")
                    nc.scalar.activation(cur, ps, ACTF.Copy)

                    # LIF scan over t on this hidden tile
                    spkt = sp.tile([P, TB], F16, tag="spk")
                    mem = mp.tile([P, BC], F32, tag="mem1")
                    nc.scalar.activation(mem, cur[:, 0:BC], ACTF.Copy)
                    nc.vector.tensor_scalar(
                        spkt[:, 0:BC], mem, THRESH, None, op0=ALU.is_gt
                    )
                    for t in range(1, T):
                        tsl = slice(t * BC, (t + 1) * BC)
                        psl = slice((t - 1) * BC, t * BC)
                        nc.vector.scalar_tensor_tensor(
                            mem, mem, BETA, spkt[:, psl],
                            op0=ALU.mult, op1=ALU.subtract,
                        )
                        nc.vector.scalar_tensor_tensor(
                            mem, mem, 1.0, cur[:, tsl], op0=ALU.bypass, op1=ALU.add
                        )
                        nc.vector.tensor_scalar(
                            spkt[:, tsl], mem, THRESH, None, op0=ALU.is_gt
                        )
                    # issue from DVE so the write needs no semaphore wait
                    nc.scalar.dma_start(spk1d[n], spkt)

                    # pace w2 gen ~one block per hidden tile, front-loaded
                    # so tiles 30/31 leave the DMA queues clean for phase B
                    if n < 2:
                        emit_w2gen(2 * n)
                        emit_w2gen(2 * n + 1)
                    elif n < 30:
                        emit_w2gen(n + 2)

                    # gen compute for tile n+2 lands after this tile's LIF
                    # ops, so LIF never waits behind DMA-blocked gen ops
                    if n + 2 < NT:
                        pend[n + 2] = emit_w1gen_compute(n + 2)

            # ---------------- Phase B: layer 2 ----------------
            with (
                tc.tile_pool(name="w2res", bufs=1) as w2r,
                tc.tile_pool(name="bspk", bufs=3) as bp,
                tc.tile_pool(name="bstate", bufs=1) as bs,
                tc.tile_pool(name="bpsum", bufs=2, space="PSUM") as bps,
            ):
                W2F = w2r.tile([P, KO2, DOUT], F16)

                acc = bs.tile([P, 2, DOUT], F32)
                mem2 = bs.tile([P, 2, DOUT], F32)
                spk2 = bs.tile([P, 2, DOUT], F32)

                first = True
                for t in range(T):
                    for h in range(2):
                        spks = bp.tile([P, KO2, P], F16, tag="spkB")
                        coff = t * BC + h * P
                        for k2 in range(KO2):
                            nc.gpsimd.dma_start(
                                spks[:, k2, :],
                                spk1d[k2, :, coff : coff + P],
                            )
                        if first:
                            # W2 residency loads ride two queues (neither
                            # shared with the spk loads) so the first
                            # blocks' matmuls aren't stream-gated.
                            for o2 in range(KO2):
                                q = nc.scalar if o2 % 2 else nc.sync
                                q.dma_start(W2F[:, o2, :], w2f16d[o2])
                            first = False
                        ps2 = bps.tile([P, DOUT], F32, tag="ps2")
                        for k2 in range(KO2):
                            for c0 in (0, 512):
                                csl = slice(c0, c0 + 512)
                                nc.tensor.matmul(
                                    ps2[:, csl], spks[:, k2, :], W2F[:, k2, csl],
                                    start=(k2 == 0), stop=(k2 == KO2 - 1),
                                )
                        m2 = mem2[:, h, :]
                        if t == 0:
                            nc.scalar.activation(m2, ps2, ACTF.Copy)
                            nc.vector.tensor_scalar(
                                acc[:, h, :], m2, THRESH, None, op0=ALU.is_gt
                            )
                            nc.scalar.activation(spk2[:, h, :], acc[:, h, :], ACTF.Copy)
                        else:
                            nc.vector.scalar_tensor_tensor(
                                m2, m2, BETA, spk2[:, h, :],
                                op0=ALU.mult, op1=ALU.subtract,
                            )
                            nc.vector.scalar_tensor_tensor(
                                m2, m2, 1.0, ps2, op0=ALU.bypass, op1=ALU.add
                            )
                            if t < T - 1:
                                nc.vector.tensor_scalar(
                                    spk2[:, h, :], m2, THRESH, None, op0=ALU.is_gt
                                )
                            nc.vector.scalar_tensor_tensor(
                                acc[:, h, :], m2, THRESH, acc[:, h, :],
                                op0=ALU.is_gt, op1=ALU.add,
                            )
                for h in range(2):
                    nc.sync.dma_start(out[h * P : (h + 1) * P, :], acc[:, h, :])

    nc.compile()
    return nc


_NC_CACHE = None


def _get_nc():
    global _NC_CACHE
    if _NC_CACHE is None:
        _NC_CACHE = _build_nc()
    return _NC_CACHE


def _make_in_maps(inputs):
    x = np.ascontiguousarray(inputs["x"], dtype=np.float32)
    shared = {
        name: np.ascontiguousarray(inputs[name], dtype=np.float32)
        for name in ("w1_mu", "w1_logvar", "eps1", "w2_mu", "w2_logvar", "eps2")
    }
    in_maps = []
    for c in range(NCORES):
        xc = x[c * BC : (c + 1) * BC]          # [BC, T, DIN]
        xtc = np.ascontiguousarray(xc.transpose(2, 1, 0)).reshape(DIN, TB)
        in_maps.append({"xt": xtc, **shared})
    return in_maps


def _run(inputs, trace=False, **kwargs):
    nc = _get_nc()
    in_maps = _make_in_maps(inputs)
    res = run_bass_kernel_spmd(
        nc, in_maps, core_ids=list(range(NCORES)), trace=trace, **kwargs
    )
    outs = [np.asarray(res.results[c]["out"]) for c in range(NCORES)]
    full = np.concatenate(outs, axis=0).astype(np.float32)
    return full, res


def kernel(**inputs):
    full, _ = _run(inputs, trace=False)
    return full

